# revision 3
# baseline (speedup 1.0000x reference)
"""Trainium2 Bass kernel v2 for nn_AutoRegerting_2954937500106.

Structure (per 8-core SPMD program):
  - Phase 0: gi0 = Wih0_slice @ x batched over all T (fp32), stored to
    DRAM and prefetched per-tick during the recurrence.
  - Recurrence: tensor-parallel gates (384 cols/core), one AllGather per
    step carrying both layers' pre-LN h chunks PLUS per-core partial LN
    stats (sum, sumsq). LayerNorm after the gather uses the gathered
    partials (no Square / stat matmuls), rsqrt via Quake bit-trick on the
    vector engine (scalar engine runs ONLY Sigmoid/Tanh -> no activation
    table thrashing). Gate matmuls in fp16 (single PE pass).
  - Head (Linear->LeakyReLU->LN->Linear(V/8)) in fp16 (h1 -> logits is
    outside the feedback loop, so fp16 noise does not amplify),
    interleaved into the recurrence in 8-step groups so it runs in the
    PE idle time during AllGather waits and keeps the PE HAM-warm.
"""
import sys as _sys
for _p in ("/opt/trn_rl_repo", "/opt/trn_rl_repo/concourse"):
    if _p not in _sys.path:
        _sys.path.append(_p)

import numpy as np
import concourse.bacc as bacc
import concourse.bass as bass
import concourse.mybir as mybir
import concourse.tile as tile

F32 = mybir.dt.float32
F16 = mybir.dt.float16
I32 = mybir.dt.int32
AF = mybir.ActivationFunctionType
ALU = mybir.AluOpType

H = 1024
E = 512
B = 16
V = 32000
NCORES = 8
KH = H // 128     # 8 h-chunks
KE = E // 128     # 4 e-chunks
MSL = 3 * 128     # 384 gate cols per core
VC = V // NCORES  # 4000
EPS = 1e-5
NEG_SLOPE = 0.01
RING = 16         # h1 history ring (steps)
GS = 8            # head group size (steps)
MAGIC = 0x5F3759DF


def _quake_rsqrt(nc, pool, out_ap, x_ap, shape, tag, niter=3):
    """out = 1/sqrt(x) elementwise via bit-trick + Newton steps (DVE only)."""
    ish = pool.tile(shape, I32, tag=f"{tag}_i", name=f"{tag}_i")
    nc.vector.tensor_scalar(ish[:], x_ap.bitcast(I32), 1, None,
                            ALU.logical_shift_right)
    nc.vector.tensor_scalar(ish[:], ish[:], -1, MAGIC, ALU.mult, ALU.add)
    a = pool.tile(shape, F32, tag=f"{tag}_a", name=f"{tag}_a")
    c = pool.tile(shape, F32, tag=f"{tag}_c", name=f"{tag}_c")
    ys = [pool.tile(shape, F32, tag=f"{tag}_y{i}", name=f"{tag}_y{i}")
          for i in range(niter - 1)]
    y = ish.bitcast(F32)
    for i in range(niter):
        nc.vector.tensor_mul(a[:], x_ap, y[:] if hasattr(y, '__getitem__') else y)
        nc.vector.tensor_mul(a[:], a[:], y[:])
        nc.vector.tensor_scalar(c[:], a[:], -0.5, 1.5, ALU.mult, ALU.add)
        dst = out_ap if i == niter - 1 else ys[i][:]
        nc.vector.tensor_mul(dst, c[:], y[:])
        if i < niter - 1:
            y = ys[i]


def build_nc(T=256, n_cores=NCORES):
    BT = T * B
    NG = T // GS  # head groups
    nc = bacc.Bacc("TRN2", target_bir_lowering=False, debug=False,
                   enable_asserts=False, num_devices=n_cores)

    xT    = nc.dram_tensor("xT",    [KE, 128, BT], F32, kind="ExternalInput").ap()
    wih0  = nc.dram_tensor("wih0",  [E, MSL], F32, kind="ExternalInput").ap()
    whh0  = nc.dram_tensor("whh0",  [H, MSL], F32, kind="ExternalInput").ap()
    wih1  = nc.dram_tensor("wih1",  [H, MSL], F32, kind="ExternalInput").ap()
    whh1  = nc.dram_tensor("whh1",  [H, MSL], F32, kind="ExternalInput").ap()
    bih0c = nc.dram_tensor("bih0c", [128, 3], F32, kind="ExternalInput").ap()
    gb0   = nc.dram_tensor("gb0",   [128, 3], F32, kind="ExternalInput").ap()
    gb1   = nc.dram_tensor("gb1",   [128, 4], F32, kind="ExternalInput").ap()
    lnw   = nc.dram_tensor("lnw",   [128, 2, KH], F32, kind="ExternalInput").ap()
    lnb   = nc.dram_tensor("lnb",   [128, 2, KH], F32, kind="ExternalInput").ap()
    lnwo  = nc.dram_tensor("lnwo",  [128, 2], F32, kind="ExternalInput").ap()
    lnbo  = nc.dram_tensor("lnbo",  [128, 2], F32, kind="ExternalInput").ap()
    eye16 = nc.dram_tensor("eye16", [16, 16], F32, kind="ExternalInput").ap()
    ln2w  = nc.dram_tensor("ln2w",  [128, KH], F32, kind="ExternalInput").ap()
    ln2b  = nc.dram_tensor("ln2b",  [128, KH], F32, kind="ExternalInput").ap()
    b1c   = nc.dram_tensor("b1c",   [128, KH], F32, kind="ExternalInput").ap()
    w1T   = nc.dram_tensor("w1T",   [H, H], F16, kind="ExternalInput").ap()
    w2cT  = nc.dram_tensor("w2cT",  [H, VC], F16, kind="ExternalInput").ap()
    out   = nc.dram_tensor("out",   [BT, VC], F32, kind="ExternalOutput").ap()

    rg = [list(range(n_cores))]
    AROWS = 2 * 128 + 4  # 260: h0 chunk, h1 chunk, stats rows (s0,q0,s1,q1)

    with tile.TileContext(nc) as tc:
        with tc.tile_pool(name="rw", bufs=1) as rw, \
             tc.tile_pool(name="ring", bufs=1) as ringp, \
             tc.tile_pool(name="dramp", bufs=1, space="DRAM") as dramp, \
             tc.tile_pool(name="rdram", bufs=3, space="DRAM") as rdram:
            gi0T = dramp.tile([3, 128, BT], F32)

            # ---------- persistent SBUF ----------
            whh0s = rw.tile([128, KH, MSL], F32)
            nc.sync.dma_start(whh0s[:], whh0.rearrange("(k p) m -> p k m", p=128))
            wih1s = rw.tile([128, KH, MSL], F32)
            nc.sync.dma_start(wih1s[:], wih1.rearrange("(k p) m -> p k m", p=128))
            whh1s = rw.tile([128, KH, MSL], F32)
            nc.sync.dma_start(whh1s[:], whh1.rearrange("(k p) m -> p k m", p=128))
            w1s = rw.tile([128, KH, H], F16)
            nc.sync.dma_start(w1s[:], w1T.rearrange("(k p) m -> p k m", p=128))
            w2s = rw.tile([128, KH, VC], F16)
            nc.sync.dma_start(w2s[:], w2cT.rearrange("(k p) v -> p k v", p=128))
            gb0s = rw.tile([128, 3], F32)
            nc.sync.dma_start(gb0s[:], gb0[:])
            gb1s = rw.tile([128, 4], F32)
            nc.sync.dma_start(gb1s[:], gb1[:])
            lnws = rw.tile([128, 2, KH], F32)
            nc.sync.dma_start(lnws[:], lnw[:])
            lnbs = rw.tile([128, 2, KH], F32)
            nc.sync.dma_start(lnbs[:], lnb[:])
            lnwos = rw.tile([128, 2], F32)
            nc.sync.dma_start(lnwos[:], lnwo[:])
            lnbos = rw.tile([128, 2], F32)
            nc.sync.dma_start(lnbos[:], lnbo[:])
            b1s = rw.tile([128, KH], F32)
            nc.sync.dma_start(b1s[:], b1c[:])
            ln2ws = rw.tile([128, KH], F32)
            nc.sync.dma_start(ln2ws[:], ln2w[:])
            ln2bs = rw.tile([128, KH], F32)
            nc.sync.dma_start(ln2bs[:], ln2b[:])
            eyes = rw.tile([16, 16], F32)
            nc.sync.dma_start(eyes[:], eye16[:])
            ones_col = rw.tile([128, 1], F32)
            nc.vector.memset(ones_col[:], 1.0)
            ones_col16 = rw.tile([128, 1], F16)
            nc.vector.memset(ones_col16[:], 1.0)
            ones_row = rw.tile([1, 128], F32)
            nc.vector.memset(ones_row[:], 1.0)
            ring_buf = ringp.tile([128, KH, RING * B], F16)  # h1 history

            # ---------- Phase 0: gi0 (fp16 matmuls, fp32 out) ----------
            with tc.tile_pool(name="p0", bufs=1) as p0, \
                 tc.tile_pool(name="ps0", bufs=2, space="PSUM") as ps0:
                wih0s = p0.tile([128, KE, MSL], F32)
                nc.sync.dma_start(wih0s[:], wih0.rearrange("(k p) m -> p k m", p=128))
                bih0s = p0.tile([128, 3], F32)
                nc.sync.dma_start(bih0s[:], bih0c[:])
                nbt = max(1, BT // 512)
                btc = BT // nbt
                for j in range(nbt):
                    xs = p0.tile([128, KE, btc], F32, tag="xs", name="xs",
                                 bufs=2)
                    nc.sync.dma_start(
                        xs[:], xT[:, :, j * btc:(j + 1) * btc].rearrange(
                            "k p n -> p k n"))
                    for m in range(3):
                        ps = ps0.tile([128, btc], F32, tag="ps", name="ps")
                        for k in range(KE):
                            nc.tensor.matmul(
                                ps[:],
                                wih0s[:, k, m * 128:(m + 1) * 128],
                                xs[:, k, :],
                                start=(k == 0), stop=(k == KE - 1))
                        gt = p0.tile([128, btc], F32, tag="gt", name="gt",
                                     bufs=2)
                        nc.scalar.activation(gt[:], ps[:], AF.Identity,
                                             bias=bih0s[:, m:m + 1])
                        nc.sync.dma_start(
                            gi0T[m, :, j * btc:(j + 1) * btc], gt[:])

            # ---------- head scheduling table ----------
            head_sched = {}

            def sched(tick, fn):
                head_sched.setdefault(tick, []).append(fn)

            gstate = {}

            def do_w1_slice(g, m):
                st = gstate.setdefault(g, {})
                if m == 0:
                    st['araw'] = hd.tile([128, KH, 128], F16, tag="araw",
                                         name="araw", bufs=2)
                    st['hstat'] = hps.tile([1, 2, 128], F32, tag="hsb",
                                           name="hstat", bufs=1)
                araw, hstat = st['araw'], st['hstat']
                s0 = (g * GS) % RING
                ps = hps.tile([128, 128], F32, tag="w1ps", name="w1ps", bufs=1)
                for kk in range(KH):
                    nc.tensor.matmul(
                        ps[:], w1s[:, kk, m * 128:(m + 1) * 128],
                        ring_buf[:, kk, s0 * B:(s0 + GS) * B],
                        start=(kk == 0), stop=(kk == KH - 1))
                t = hd.tile([128, 128], F16, tag="hw1t", name="hw1t", bufs=2)
                nc.scalar.activation(t[:], ps[:], AF.Identity,
                                     bias=b1s[:, m:m + 1])
                u = hd.tile([128, 128], F16, tag="hw1u", name="hw1u", bufs=2)
                nc.vector.tensor_scalar_mul(u[:], t[:], NEG_SLOPE)
                stg = hd.tile([128, 2, 128], F16, tag="hstg", name="hstg",
                              bufs=2)
                nc.vector.tensor_max(stg[:, 0], t[:], u[:])
                nc.vector.tensor_mul(stg[:, 1], stg[:, 0], stg[:, 0])
                nc.vector.tensor_copy(araw[:, m], stg[:, 0])
                nc.tensor.matmul(hstat[:], ones_col16[:], stg[:],
                                 start=(m == 0), stop=(m == KH - 1))

            def do_ln2(g):
                st = gstate[g]
                araw, hstat = st['araw'], st['hstat']
                mr2 = hd.tile([1, 2, 128], F32, tag="mr2", name="mr2", bufs=2)
                nc.vector.tensor_scalar_mul(mr2[:, 0], hstat[:, 0], 1.0 / H)
                u2 = hd.tile([1, 128], F32, tag="u2", name="u2", bufs=2)
                nc.vector.tensor_scalar(u2[:], hstat[:, 1], 1.0 / H, EPS,
                                        ALU.mult, ALU.add)
                t2 = hd.tile([1, 128], F32, tag="t2", name="t2", bufs=2)
                nc.vector.tensor_mul(t2[:], mr2[:, 0], mr2[:, 0])
                nc.vector.tensor_sub(t2[:], u2[:], t2[:])
                _quake_rsqrt(nc, hd, mr2[:, 1], t2[:], [1, 128], "hq", niter=2)
                hbc = hps.tile([128, 2, 128], F32, tag="hsb", name="hbc", bufs=1)
                nc.tensor.matmul(hbc[:], ones_row[:], mr2[:], start=True,
                                 stop=True)
                bc2 = hd.tile([128, 2, 128], F16, tag="bc2", name="bc2", bufs=2)
                nc.vector.tensor_copy(bc2[:], hbc[:])
                aT = hd.tile([128, KH, 128], F16, tag="aT", name="aT", bufs=2)
                st['aT'] = aT
                for m in range(KH):
                    v = hd.tile([128, 128], F16, tag="hnrm", name="hnrm", bufs=2)
                    nc.vector.tensor_sub(v[:], araw[:, m], bc2[:, 0])
                    nc.vector.tensor_mul(v[:], v[:], bc2[:, 1])
                    nc.vector.tensor_scalar(aT[:, m], v[:], ln2ws[:, m:m + 1],
                                            ln2bs[:, m:m + 1], ALU.mult, ALU.add)

            def do_w2_slice(g, v):
                st = gstate[g]
                aT = st['aT']
                ps = hps.tile([128, 500], F32, tag="w2ps", name="w2ps", bufs=2)
                for kk in range(KH):
                    nc.tensor.matmul(ps[:], aT[:, kk],
                                     w2s[:, kk, v * 500:(v + 1) * 500],
                                     start=(kk == 0), stop=(kk == KH - 1))
                lg = hd.tile([128, 500], F32, tag="lg", name="lg", bufs=2)
                if v % 2 == 0:
                    nc.vector.tensor_copy(lg[:], ps[:])
                else:
                    nc.scalar.copy(lg[:], ps[:])
                nc.sync.dma_start(
                    out[g * 128:(g + 1) * 128, v * 500:(v + 1) * 500], lg[:])
                if v == KH - 1:
                    gstate.pop(g)

            for g in range(NG):
                base = g * GS + 10
                for m in range(KH):
                    sched(base + m, (do_w1_slice, g, m))
                sched(base + KH - 1, (do_ln2, g))
                for v in range(KH):
                    sched(base + KH + v, (do_w2_slice, g, v))

            # ---------- recurrence + interleaved head ----------
            with tc.tile_pool(name="rs", bufs=2) as rs, \
                 tc.tile_pool(name="rtmp", bufs=3) as rtmp, \
                 tc.tile_pool(name="hd", bufs=2) as hd, \
                 tc.tile_pool(name="rps", bufs=1, space="PSUM") as rps, \
                 tc.tile_pool(name="hps", bufs=1, space="PSUM") as hps:

                prev_hout = None
                agout = None
                gi0_tiles = {}

                def prefetch_gi0(kk):
                    tgi_ = min(kk, T - 1)
                    gt_ = rs.tile([128, 3, B], F32, tag="gi0t", name="gi0t",
                                  bufs=3)
                    nc.scalar.dma_start(
                        gt_[:], gi0T[:, :, tgi_ * B:(tgi_ + 1) * B].rearrange(
                            "m p b -> p m b"))
                    gi0_tiles[kk] = gt_

                prefetch_gi0(0)
                prefetch_gi0(1)

                for k in range(T + 2):
                    last = (k == T + 1)
                    rslot = (k - 2) % RING
                    ring_l1 = ring_buf[:, :, rslot * B:(rslot + 1) * B]
                    hn1 = rs.tile([128, KH, B], F32, tag="hn1", name="hn1")

                    hn0 = rs.tile([128, KH, B], F32, tag="hn0", name="hn0")
                    if k == 0:
                        nc.vector.memset(hn0[:], 0.0)
                        nc.vector.memset(hn1[:], 0.0)
                        nc.vector.memset(ring_l1, 0.0)
                        bcs = None
                    else:
                        ag3 = agout.rearrange("(c r) b -> c r b", c=n_cores)
                        hp = rs.tile([128, 2, KH, B], F32, tag="hp", name="hp")
                        hv = ag3[:, 0:256].rearrange("c (l p) b -> p l c b", l=2)
                        nc.sync.dma_start(hp[:, 0], hv[:, 0])
                        nc.sync.dma_start(hp[:, 1], hv[:, 1])
                        sraw = rtmp.tile([1, n_cores, 4, B], F32, tag="sraw",
                                         name="sraw", bufs=2)
                        nc.scalar.dma_start(sraw[:],
                                            ag3[:, 256:260].unsqueeze(0))
                        ssum = rtmp.tile([1, 4, B], F32, tag="ssum", name="ssum")
                        nc.vector.tensor_reduce(
                            ssum[:], sraw.rearrange("o c s b -> o s b c"),
                            mybir.AxisListType.X, ALU.add)
                        s4 = ssum.rearrange("o (l t) b -> o l t b", t=2)
                        mstat = rtmp.tile([1, 2, 2, B], F32, tag="mstat",
                                          name="mstat")
                        nc.vector.tensor_scalar_mul(mstat[:, 0], s4[:, :, 0],
                                                    1.0 / H)
                        uq = rtmp.tile([1, 2, B], F32, tag="uq", name="uq")
                        nc.vector.tensor_scalar(uq[:], s4[:, :, 1], 1.0 / H,
                                                EPS, ALU.mult, ALU.add)
                        vq = rtmp.tile([1, 2, B], F32, tag="vq", name="vq")
                        nc.vector.tensor_mul(vq[:], mstat[:, 0], mstat[:, 0])
                        nc.vector.tensor_sub(vq[:], uq[:], vq[:])
                        _quake_rsqrt(nc, rtmp, mstat[:, 1], vq[:], [1, 2, B],
                                     "rq", niter=3)
                        bcp = rps.tile([128, 2, 2, B], F32, tag="paux",
                                       name="bcp")
                        nc.tensor.matmul(bcp[:], ones_row[:], mstat[:],
                                         start=True, stop=True)
                        bcs = rtmp.tile([128, 2, 2, B], F32, tag="bcs",
                                        name="bcs")
                        nc.vector.tensor_copy(bcs[:], bcp[:])
                        # normalize layer 0 -> hnb0 (fp16)
                        nt = rtmp.tile([128, KH, B], F32, tag="nt", name="nt", bufs=1)
                        nc.vector.tensor_sub(
                            nt[:], hp[:, 0],
                            bcs[:, 0, 0].unsqueeze(1).broadcast_to([128, KH, B]))
                        nc.vector.tensor_mul(
                            nt[:], nt[:],
                            bcs[:, 1, 0].unsqueeze(1).broadcast_to([128, KH, B]))
                        nc.vector.tensor_mul(
                            nt[:], nt[:],
                            lnws[:, 0].unsqueeze(2).broadcast_to([128, KH, B]))
                        nc.vector.tensor_add(
                            hn0[:], nt[:],
                            lnbs[:, 0].unsqueeze(2).broadcast_to([128, KH, B]))
                        # normalize layer 1 -> ring slot (fp16)
                        if k == 1:
                            nc.vector.memset(hn1[:], 0.0)
                            nc.vector.memset(ring_l1, 0.0)
                        else:
                            nt1 = rtmp.tile([128, KH, B], F32, tag="nt1",
                                            name="nt1", bufs=1)
                            nc.vector.tensor_sub(
                                nt1[:], hp[:, 1],
                                bcs[:, 0, 1].unsqueeze(1).broadcast_to(
                                    [128, KH, B]))
                            nc.vector.tensor_mul(
                                nt1[:], nt1[:],
                                bcs[:, 1, 1].unsqueeze(1).broadcast_to(
                                    [128, KH, B]))
                            nc.vector.tensor_mul(
                                nt1[:], nt1[:],
                                lnws[:, 1].unsqueeze(2).broadcast_to(
                                    [128, KH, B]))
                            nc.vector.tensor_add(
                                hn1[:], nt1[:],
                                lnbs[:, 1].unsqueeze(2).broadcast_to(
                                    [128, KH, B]))
                            nc.vector.tensor_copy(ring_l1, hn1[:])
                    if last:
                        for fn in head_sched.pop(k, []):
                            fn[0](*fn[1:])
                        break

                    # ---- gate matmuls (h-stationary fp16) ----
                    pA0 = rps.tile([16, MSL], F32, tag="pA0", name="pA0")
                    pA1i = rps.tile([16, MSL], F32, tag="pA1i", name="pA1i")
                    pA1h = rps.tile([16, MSL], F32, tag="pA1h", name="pA1h")
                    for kk in range(KH):
                        nc.tensor.matmul(pA0[:], hn0[:, kk, :],
                                         whh0s[:, kk, :],
                                         start=(kk == 0), stop=(kk == KH - 1))
                    for kk in range(KH):
                        nc.tensor.matmul(pA1i[:], hn0[:, kk, :],
                                         wih1s[:, kk, :],
                                         start=(kk == 0), stop=(kk == KH - 1))
                    for kk in range(KH):
                        nc.tensor.matmul(pA1h[:], hn1[:, kk, :],
                                         whh1s[:, kk, :],
                                         start=(kk == 0), stop=(kk == KH - 1))
                    sA0 = rtmp.tile([16, MSL], F32, tag="sA0", name="sA0", bufs=1)
                    nc.scalar.copy(sA0[:], pA0[:])
                    sA1i = rtmp.tile([16, MSL], F32, tag="sA1i", name="sA1i", bufs=1)
                    nc.vector.tensor_copy(sA1i[:], pA1i[:])
                    sA1h = rtmp.tile([16, MSL], F32, tag="sA1h", name="sA1h", bufs=1)
                    nc.scalar.copy(sA1h[:], pA1h[:])
                    sA1rz = rtmp.tile([16, 256], F32, tag="sA1rz", name="sA1rz", bufs=1)
                    nc.vector.tensor_add(sA1rz[:], sA1i[:, 0:256],
                                         sA1h[:, 0:256])
                    pT = rps.tile([128, 7, B], F32, tag="paux", name="pT")
                    nc.tensor.transpose(pT[:, 0], sA0[:, 0:128], eyes[:])
                    nc.tensor.transpose(pT[:, 1], sA0[:, 128:256], eyes[:])
                    nc.tensor.transpose(pT[:, 2], sA0[:, 256:384], eyes[:])
                    nc.tensor.transpose(pT[:, 3], sA1i[:, 256:384], eyes[:])
                    nc.tensor.transpose(pT[:, 4], sA1rz[:, 0:128], eyes[:])
                    nc.tensor.transpose(pT[:, 5], sA1rz[:, 128:256], eyes[:])
                    nc.tensor.transpose(pT[:, 6], sA1h[:, 256:384], eyes[:])

                    gi0t = gi0_tiles.pop(k)
                    if k + 2 <= T:
                        prefetch_gi0(k + 2)

                    hout = rtmp.tile([128, 2, B], F32, tag="hout", name="hout")
                    # ---- layer0 gates ----
                    t0r = rtmp.tile([128, B], F32, tag="t0r", name="t0r")
                    nc.vector.scalar_tensor_tensor(t0r[:], pT[:, 0],
                                                   gb0s[:, 0:1], gi0t[:, 0],
                                                   ALU.add, ALU.add)
                    r0 = rtmp.tile([128, B], F32, tag="r0", name="r0")
                    nc.scalar.activation(r0[:], t0r[:], AF.Sigmoid)
                    t0z = rtmp.tile([128, B], F32, tag="t0z", name="t0z")
                    nc.vector.scalar_tensor_tensor(t0z[:], pT[:, 1],
                                                   gb0s[:, 1:2], gi0t[:, 1],
                                                   ALU.add, ALU.add)
                    z0 = rtmp.tile([128, B], F32, tag="z0", name="z0")
                    nc.scalar.activation(z0[:], t0z[:], AF.Sigmoid)
                    hn0m = rtmp.tile([128, B], F32, tag="hn0m", name="hn0m")
                    nc.vector.tensor_scalar_add(hn0m[:], pT[:, 2], gb0s[:, 2:3])
                    nc.vector.tensor_mul(hn0m[:], hn0m[:], r0[:])
                    nc.vector.tensor_add(hn0m[:], hn0m[:], gi0t[:, 2])
                    n0 = rtmp.tile([128, B], F32, tag="n0", name="n0")
                    nc.scalar.activation(n0[:], hn0m[:], AF.Tanh)
                    hc0 = rtmp.tile([128, B], F32, tag="hc0", name="hc0")
                    if k == 0:
                        nc.vector.memset(hc0[:], 0.0)
                    else:
                        nc.vector.tensor_sub(hc0[:], prev_hout[:, 0],
                                             bcs[:, 0, 0])
                        nc.vector.tensor_mul(hc0[:], hc0[:], bcs[:, 1, 0])
                        nc.vector.tensor_scalar_mul(hc0[:], hc0[:],
                                                    lnwos[:, 0:1])
                        nc.vector.tensor_scalar_add(hc0[:], hc0[:],
                                                    lnbos[:, 0:1])
                    nc.vector.tensor_sub(hc0[:], hc0[:], n0[:])
                    nc.vector.tensor_mul(hc0[:], hc0[:], z0[:])
                    nc.vector.tensor_add(hout[:, 0], hc0[:], n0[:])

                    # ---- layer1 gates ----
                    r1 = rtmp.tile([128, B], F32, tag="r1", name="r1")
                    nc.scalar.activation(r1[:], pT[:, 4], AF.Sigmoid,
                                         bias=gb1s[:, 0:1])
                    z1 = rtmp.tile([128, B], F32, tag="z1", name="z1")
                    nc.scalar.activation(z1[:], pT[:, 5], AF.Sigmoid,
                                         bias=gb1s[:, 1:2])
                    hn1m = rtmp.tile([128, B], F32, tag="hn1m", name="hn1m")
                    nc.vector.tensor_scalar_add(hn1m[:], pT[:, 6], gb1s[:, 3:4])
                    nc.vector.tensor_mul(hn1m[:], hn1m[:], r1[:])
                    nc.vector.scalar_tensor_tensor(hn1m[:], pT[:, 3],
                                                   gb1s[:, 2:3], hn1m[:],
                                                   ALU.add, ALU.add)
                    n1 = rtmp.tile([128, B], F32, tag="n1", name="n1")
                    nc.scalar.activation(n1[:], hn1m[:], AF.Tanh)
                    hc1 = rtmp.tile([128, B], F32, tag="hc1", name="hc1")
                    if k <= 1:
                        nc.vector.memset(hc1[:], 0.0)
                    else:
                        nc.vector.tensor_sub(hc1[:], prev_hout[:, 1],
                                             bcs[:, 0, 1])
                        nc.vector.tensor_mul(hc1[:], hc1[:], bcs[:, 1, 1])
                        nc.vector.tensor_scalar_mul(hc1[:], hc1[:],
                                                    lnwos[:, 1:2])
                        nc.vector.tensor_scalar_add(hc1[:], hc1[:],
                                                    lnbos[:, 1:2])
                    nc.vector.tensor_sub(hc1[:], hc1[:], n1[:])
                    nc.vector.tensor_mul(hc1[:], hc1[:], z1[:])
                    nc.vector.tensor_add(hout[:, 1], hc1[:], n1[:])

                    # ---- partial LN stats + AllGather ----
                    sq0 = rtmp.tile([128, B], F32, tag="sq0", name="sq0")
                    nc.vector.tensor_mul(sq0[:], hout[:, 0], hout[:, 0])
                    sq1 = rtmp.tile([128, B], F32, tag="sq1", name="sq1")
                    nc.vector.tensor_mul(sq1[:], hout[:, 1], hout[:, 1])
                    csp = rps.tile([1, 4, B], F32, tag="paux", name="csp")
                    nc.tensor.matmul(csp[:, 0], ones_col[:], hout[:, 0],
                                     start=True, stop=True)
                    nc.tensor.matmul(csp[:, 1], ones_col[:], sq0[:],
                                     start=True, stop=True)
                    nc.tensor.matmul(csp[:, 2], ones_col[:], hout[:, 1],
                                     start=True, stop=True)
                    nc.tensor.matmul(csp[:, 3], ones_col[:], sq1[:],
                                     start=True, stop=True)
                    cs_sb = rtmp.tile([1, 4, B], F32, tag="cs_sb", name="cs_sb")
                    nc.vector.tensor_copy(cs_sb[:], csp[:])

                    agin = rdram.tile([AROWS, B], F32, tag="agin", name="agin")
                    nc.sync.dma_start(
                        agin[0:256].rearrange("(l p) b -> p l b", p=128),
                        hout[:])
                    nc.scalar.dma_start(
                        agin[256:260].unsqueeze(0), cs_sb[:])
                    agout = rdram.tile([n_cores * AROWS, B], F32, tag="agout",
                                       name="agout", addr_space="Shared")
                    nc.gpsimd.collective_compute(
                        "AllGather", ALU.bypass, replica_groups=rg,
                        ins=[agin.opt()], outs=[agout.opt()])

                    prev_hout = hout

                    # ---- interleaved head work for this tick ----
                    for fn in head_sched.pop(k, []):
                        fn[0](*fn[1:])

                # ---- head tail (ticks beyond T+1) ----
                for t in sorted(head_sched.keys()):
                    for fn in head_sched.pop(t):
                        fn[0](*fn[1:])
    return nc


# ===================== host-side prep / post =====================

def _np(x):
    return np.asarray(x)


def prep_in_maps(inputs, T=256, n_cores=NCORES):
    ids = _np(inputs['input']).astype(np.int64)[:, :T]
    embd = _np(inputs['embd']).astype(np.float32)
    BT = T * B
    x = embd[ids]                                               # [B, T, E]
    xT = np.ascontiguousarray(
        x.transpose(2, 1, 0).reshape(E, T * B)).astype(np.float32)
    xT = xT.reshape(KE, 128, BT)

    def gate_slice(W, c):
        cols = []
        for g in range(3):
            cols.append(W[g * H + c * 128:(g * H + (c + 1) * 128), :])
        Wc = np.concatenate(cols, axis=0)
        return np.ascontiguousarray(Wc.T).astype(np.float32)

    def bias_slice(b, c, g):
        return b[g * H + c * 128:g * H + (c + 1) * 128]

    lnw = np.stack([_np(inputs['ln0_w']), _np(inputs['ln1_w'])], 0)
    lnb = np.stack([_np(inputs['ln0_b']), _np(inputs['ln1_b'])], 0)
    lnw_t = np.ascontiguousarray(
        lnw.reshape(2, KH, 128).transpose(2, 0, 1)).astype(np.float32)
    lnb_t = np.ascontiguousarray(
        lnb.reshape(2, KH, 128).transpose(2, 0, 1)).astype(np.float32)
    ln2w_t = np.ascontiguousarray(
        _np(inputs['ln2_w']).reshape(KH, 128).T).astype(np.float32)
    ln2b_t = np.ascontiguousarray(
        _np(inputs['ln2_b']).reshape(KH, 128).T).astype(np.float32)
    b1_t = np.ascontiguousarray(
        _np(inputs['b1']).reshape(KH, 128).T).astype(np.float32)
    w1T = np.ascontiguousarray(
        _np(inputs['W1']).astype(np.float32).T).astype(np.float16)
    W2 = _np(inputs['W2']).astype(np.float32)

    Wih0 = _np(inputs['Wih0']).astype(np.float32)
    Whh0 = _np(inputs['Whh0']).astype(np.float32)
    Wih1 = _np(inputs['Wih1']).astype(np.float32)
    Whh1 = _np(inputs['Whh1']).astype(np.float32)
    bih0 = _np(inputs['bih0']).astype(np.float32)
    bhh0 = _np(inputs['bhh0']).astype(np.float32)
    bih1 = _np(inputs['bih1']).astype(np.float32)
    bhh1 = _np(inputs['bhh1']).astype(np.float32)

    in_maps = []
    for c in range(n_cores):
        bih0c = np.stack([bias_slice(bih0, c, g) for g in range(3)], 1)
        gb0 = np.stack([bias_slice(bhh0, c, g) for g in range(3)], 1)
        gb1 = np.stack([
            bias_slice(bih1, c, 0) + bias_slice(bhh1, c, 0),
            bias_slice(bih1, c, 1) + bias_slice(bhh1, c, 1),
            bias_slice(bih1, c, 2),
            bias_slice(bhh1, c, 2)], 1)
        lnw_o = np.ascontiguousarray(
            lnw[:, c * 128:(c + 1) * 128].T).astype(np.float32)   # [128, 2]
        lnb_o = np.ascontiguousarray(
            lnb[:, c * 128:(c + 1) * 128].T).astype(np.float32)
        eye16 = np.eye(16, dtype=np.float32)
        w2cT = np.ascontiguousarray(
            W2[c * VC:(c + 1) * VC, :].T).astype(np.float16)
        in_maps.append({
            'xT': xT, 'wih0': gate_slice(Wih0, c), 'whh0': gate_slice(Whh0, c),
            'wih1': gate_slice(Wih1, c), 'whh1': gate_slice(Whh1, c),
            'bih0c': np.ascontiguousarray(bih0c).astype(np.float32),
            'gb0': np.ascontiguousarray(gb0).astype(np.float32),
            'gb1': np.ascontiguousarray(gb1).astype(np.float32),
            'lnw': lnw_t, 'lnb': lnb_t, 'lnwo': lnw_o, 'lnbo': lnb_o,
            'eye16': eye16,
            'ln2w': ln2w_t, 'ln2b': ln2b_t, 'b1c': b1_t,
            'w1T': w1T, 'w2cT': w2cT,
        })
    return in_maps


def postprocess(results, inputs, T=256):
    b2 = _np(inputs['b2']).astype(np.float32)
    full = np.concatenate([r['out'] for r in results], axis=1)  # [BT, V]
    full = full.reshape(T, B, V).transpose(1, 0, 2)             # [B, T, V]
    return full + b2


# ===================== numpy mirror (for sim testing) =====================

def numpy_reference(inputs, T=256):
    ids = _np(inputs['input']).astype(np.int64)[:, :T]
    embd = _np(inputs['embd'])
    x = embd[ids].astype(np.float32)
    h0 = np.zeros((B, H), np.float32)
    h1 = np.zeros((B, H), np.float32)

    def ln(v, w, bb):
        m = v.mean(-1, keepdims=True)
        var = v.var(-1, keepdims=True)
        return (v - m) / np.sqrt(var + EPS) * w + bb

    def gru(xx, hh, Wih, Whh, bih, bhh):
        gi = xx @ _np(Wih).T + _np(bih)
        gh = hh @ _np(Whh).T + _np(bhh)
        ir, iz, inn = np.split(gi, 3, -1)
        hr, hz, hn_ = np.split(gh, 3, -1)
        r = 1 / (1 + np.exp(-(ir + hr)))
        z = 1 / (1 + np.exp(-(iz + hz)))
        n = np.tanh(inn + r * hn_)
        return (1 - z) * n + z * hh

    outs = []
    for t in range(T):
        h0 = ln(gru(x[:, t], h0, inputs['Wih0'], inputs['Whh0'],
                    inputs['bih0'], inputs['bhh0']),
                _np(inputs['ln0_w']), _np(inputs['ln0_b']))
        h1 = ln(gru(h0, h1, inputs['Wih1'], inputs['Whh1'],
                    inputs['bih1'], inputs['bhh1']),
                _np(inputs['ln1_w']), _np(inputs['ln1_b']))
        a = h1 @ _np(inputs['W1']).T + _np(inputs['b1'])
        a = np.where(a > 0, a, NEG_SLOPE * a)
        a = ln(a, _np(inputs['ln2_w']), _np(inputs['ln2_b']))
        outs.append(a @ _np(inputs['W2']).T + _np(inputs['b2']))
    return np.stack(outs, 1)  # [B, T, V]


# ===================== NEFF disk cache =====================

def _install_neff_cache():
    import hashlib, os, shutil
    import concourse.bass2jax as b2j
    from concourse.bass_utils import compile_bir_kernel as _real
    if getattr(b2j, "_ant_neff_cache_installed", False):
        return
    cache_dir = os.path.expanduser("~/.cache/bass_neff_cache")
    os.makedirs(cache_dir, exist_ok=True)

    def cached(bir_json, tmpdir, neff_name="file.neff"):
        key = hashlib.sha256(bir_json).hexdigest()
        p = os.path.join(cache_dir, key + ".neff")
        out = os.path.join(tmpdir, neff_name)
        if os.path.exists(p):
            shutil.copyfile(p, out)
            return out
        r = _real(bir_json, tmpdir, neff_name)
        try:
            shutil.copyfile(r, p)
        except OSError:
            pass
        return r

    b2j.compile_bir_kernel = cached
    b2j._ant_neff_cache_installed = True


# ===================== NTFF profile shim (for traced runs) ==================

def _install_axon_prof():
    import types, ctypes, contextlib
    try:
        from antenv import axon_hooks  # noqa: F401
        return
    except ImportError:
        pass
    so_path = "/opt/axon/libaxon_pjrt.so"
    try:
        lib = ctypes.CDLL(so_path)
    except OSError:
        return
    hook = None
    if hasattr(lib, "axon_start_nrt_profile"):
        lib.axon_start_nrt_profile.argtypes = [
            ctypes.POINTER(ctypes.c_int64), ctypes.c_size_t]
        lib.axon_start_nrt_profile.restype = ctypes.c_int64
        lib.axon_stop_nrt_profile.argtypes = [ctypes.c_char_p]
        lib.axon_stop_nrt_profile.restype = ctypes.c_int64

        @contextlib.contextmanager
        def hook(output_dir, device_ids):
            import jax
            jax.devices()
            if device_ids:
                ids = (ctypes.c_int64 * len(device_ids))(*device_ids)
                rc = lib.axon_start_nrt_profile(ids, len(device_ids))
            else:
                rc = lib.axon_start_nrt_profile(None, 0)
            if rc != 0:
                raise RuntimeError(f"axon_start_nrt_profile rc={rc}")
            try:
                yield
            finally:
                lib.axon_stop_nrt_profile(str(output_dir).encode())

    mod = types.ModuleType("antenv.axon_hooks")
    _h = [hook]
    mod.set_axon_ntff_profile_hook = lambda h: _h.__setitem__(0, h)
    mod.get_axon_ntff_profile_hook = lambda: _h[0]
    _sys.modules["antenv.axon_hooks"] = mod
    import antenv
    antenv.axon_hooks = mod


# ===================== entry point =====================

_NC = None


def _get_nc():
    global _NC
    if _NC is None:
        _install_neff_cache()
        nc = build_nc(T=256)
        nc.compile()
        _NC = nc
    return _NC


def kernel(**inputs):
    import numpy as np
    from concourse import bass_utils
    nc = _get_nc()
    in_maps = prep_in_maps(inputs, T=256)
    res = bass_utils.run_bass_kernel_spmd(
        nc, in_maps, core_ids=list(range(NCORES)))
    return postprocess(res.results, inputs, T=256)


def kernel_traced(**inputs):
    from concourse import bass_utils
    _install_axon_prof()
    nc = _get_nc()
    in_maps = prep_in_maps(inputs, T=256)
    res = bass_utils.run_bass_kernel_spmd(
        nc, in_maps, core_ids=list(range(NCORES)), trace=True)
    return postprocess(res.results, inputs, T=256), res.exec_time_ns
